# revision 19
# baseline (speedup 1.0000x reference)
"""Trainium2 Bass kernel for nn_MixedChunkAttentionLayer.

Sharding: pure data-parallel over batch — B=8 batches onto 8 NeuronCores,
one batch per core, zero cross-core communication.

Per-core pipeline (batch b, C=256, T=8192, G=128, QK=128, HID=512):
  - InstanceNorm is folded into the projections: bn_stats/bn_aggr + Newton
    rsqrt give rstd r and mean mu per channel; the kernel scales
    Wg/Wqk by r on-device (Wg_s = r*Wg) and computes per-feature bias
    K = (-mu*r)^T W_s via tiny matmuls, so the projections consume the
    RAW bf16 q pieces as the moving operand and the silu ACT applies the
    bias: gate = silu(Wg_s^T q + K). No qn materialization pass.
  - v arrives host-premasked (v*m0, m0=1 where mask==0) which absorbs both
    the quadratic attn j-mask and the linear lin_k n-mask, so vm needs no
    per-group scale and the OffsetScale gammas fold into qsA/qsB:
      qsA = qkT*(g0*g2/G), qsB = qkT*(g1*g3/T)
  - The linear branch collapses into the quadratic one (R = laplace(sim)+S);
    sim and S for each group are produced by ONE matmul with the moving
    operand [qsA_g | qsB_g] packed [128,256], psum [128,1024] per supertile.
  - laplace chain engineered for engine balance: the two PSUM reads run on
    ACT (zzr = D1*sim+D2 -> bf16) and DVE (stl = S+0.5 -> bf16); the rest
    are cheap all-SBUF bf16 DVE ops (w = zzr^2, tt2 = (1+w)*zzr,
    R = 0.5*tanh(AB*tt2)+stl) eligible for the DVE 2x/4x fast modes.
  - z^T[ec] = vm_g[:,ec]^T @ R_g; z = z^T * gateT; out^T = Wo^T z, * m1,
    stored bf16 (host upcasts to f32).

laplace_attn(x) = Phi((x-mu)/sigma) is evaluated as
  0.5*(1 + tanh(zz*(a + b*zz^2))), zz=(x-mu)/sigma
(max abs err 1.8e-4) so every ACT function used (Silu/Tanh/Identity)
lives in the single `silu_and_others` table set — no table reloads.
"""

import math
import sys

if "/opt/trn_rl_repo" not in sys.path:
    sys.path.insert(0, "/opt/trn_rl_repo")

import numpy as np
import ml_dtypes

B, C, T = 8, 256, 8192
G = 128
QK = 128
HID = 512
NG = T // G          # 64 groups
ST = 512             # supertile token count
NST = T // ST        # 16 supertiles
GPS = ST // G        # 4 groups per supertile
NCC = C // 128       # 2 contraction chunks
NHC = HID // 128     # 4 HID chunks
NOC = C // 128       # 2 output-channel chunks

MU_L = math.sqrt(0.5)
STD_L = math.sqrt(0.25 * math.pi)
S1_L = 1.0 / STD_L           # zz = S1*x + C1
C1_L = -MU_L / STD_L
A_C = math.sqrt(2.0 / math.pi)
B_C = A_C * 0.044715
# zzr = D1*x + D2 = sqrt(B/A)*zz ; w = zzr^2 = (B/A)*zz^2
# tanh arg = A*zz*(1+w) = AB_SCALE * (1+w)*zzr
_RBA = math.sqrt(B_C / A_C)
D1_L = S1_L * _RBA
D2_L = C1_L * _RBA
AB_SCALE = A_C / _RBA

_PROG = None  # cached — program is input-independent


def _build_program():
    import concourse.bass as bass
    import concourse.tile as tile
    from concourse import bacc, mybir

    f32 = mybir.dt.float32
    bf16 = mybir.dt.bfloat16
    i32 = mybir.dt.int32
    AF = mybir.ActivationFunctionType
    OP = mybir.AluOpType

    nc = bacc.Bacc("TRN2", target_bir_lowering=False, debug=False, num_devices=8)

    q_d = nc.dram_tensor("q", [C, T], bf16, kind="ExternalInput")
    v_d = nc.dram_tensor("v", [C, T], bf16, kind="ExternalInput")
    wg_d = nc.dram_tensor("wg", [C, HID], bf16, kind="ExternalInput")
    wv_d = nc.dram_tensor("wv", [C, HID], bf16, kind="ExternalInput")
    wqk_d = nc.dram_tensor("wqk", [C, QK], bf16, kind="ExternalInput")
    wo_d = nc.dram_tensor("wo", [HID, C], bf16, kind="ExternalInput")
    gA_d = nc.dram_tensor("gA", [QK, 1], f32, kind="ExternalInput")
    gB_d = nc.dram_tensor("gB", [QK, 1], f32, kind="ExternalInput")
    out_d = nc.dram_tensor("out", [C, T], f32, kind="ExternalOutput")

    with tile.TileContext(nc) as tc:
        with (
            tc.tile_pool(name="const", bufs=1) as p_const,
            tc.tile_pool(name="qstage", bufs=1) as p_qstage,
            tc.tile_pool(name="stats", bufs=2) as p_stats,
            tc.tile_pool(name="vstage", bufs=16) as p_vstage,
            tc.tile_pool(name="stw", bufs=2) as p_st,          # within-supertile
            tc.tile_pool(name="stx", bufs=8) as p_stx,         # live 2 supertiles
            tc.tile_pool(name="lap", bufs=2) as p_lap,         # laplace temps
            tc.tile_pool(name="carry", bufs=2) as p_carry,     # R across phases
            tc.tile_pool(name="outp", bufs=2) as p_out,
            tc.tile_pool(name="psA", bufs=4, space="PSUM") as psA,
            tc.tile_pool(name="psAttn", bufs=1, space="PSUM") as psAttn,
            tc.tile_pool(name="psZ", bufs=2, space="PSUM") as psZ,
        ):
            # ---------------- constants ----------------
            # weight loads go on the gpsimd HWDGE queue so the sync queue is
            # free for the v supertile loads the PE prologue depends on;
            # wv first — the vh prelude needs it.
            wg_sb = []
            wv_sb = []
            wqk_sb = []
            for cc in range(NCC):
                t_ = p_const.tile([128, HID], bf16, tag=f"wv{cc}", name=f"wv{cc}")
                nc.gpsimd.dma_start(out=t_, in_=wv_d[cc * 128:(cc + 1) * 128, :])
                wv_sb.append(t_)
            for cc in range(NCC):
                t_ = p_const.tile([128, HID], bf16, tag=f"wg{cc}", name=f"wg{cc}")
                nc.gpsimd.dma_start(out=t_, in_=wg_d[cc * 128:(cc + 1) * 128, :])
                wg_sb.append(t_)
                t_ = p_const.tile([128, QK], bf16, tag=f"wqk{cc}", name=f"wqk{cc}")
                nc.gpsimd.dma_start(out=t_, in_=wqk_d[cc * 128:(cc + 1) * 128, :])
                wqk_sb.append(t_)
            wo_sb = []
            for hc in range(NHC):
                t_ = p_const.tile([128, C], bf16, tag=f"wo{hc}", name=f"wo{hc}")
                nc.gpsimd.dma_start(out=t_, in_=wo_d[hc * 128:(hc + 1) * 128, :])
                wo_sb.append(t_)
            gA_sb = p_const.tile([QK, 1], f32, tag="gA")
            nc.gpsimd.dma_start(out=gA_sb, in_=gA_d[:, :])
            gB_sb = p_const.tile([QK, 1], f32, tag="gB")
            nc.gpsimd.dma_start(out=gB_sb, in_=gB_d[:, :])
            bias_d2 = p_const.tile([128, 1], f32, tag="bias_d2")
            nc.vector.memset(bias_d2, D2_L)
            bias_one = p_const.tile([128, 1], f32, tag="bias_one")
            nc.vector.memset(bias_one, 1.0)

            # ---------------- q staging + stats ----------------
            # q streamed in [128, QP] pieces; the SAME pieces stay resident as
            # the projections' moving operand (instance norm is folded into
            # the weights), so q is read from HBM exactly once.
            NQP = 4
            QP = T // NQP

            def emit_q_stats():
                qpieces = []
                statst = []
                for cc in range(NCC):
                    pieces = []
                    stats = p_stats.tile([128, T // 512, 6], f32,
                                         tag=f"bnstats{cc}", name="stats")
                    for p in range(NQP):
                        qf = p_qstage.tile([128, QP], bf16, tag=f"qf{cc}{p}",
                                           name="qf", bufs=1)
                        # ACT's HWDGE queue — dispatches in parallel with the
                        # sync queue's weight/v loads
                        nc.scalar.dma_start(
                            out=qf,
                            in_=q_d[cc * 128:(cc + 1) * 128,
                                    p * QP:(p + 1) * QP],
                        )
                        qfv = qf.rearrange("p (n f) -> p n f", f=512)
                        for n in range(QP // 512):
                            nc.vector.bn_stats(
                                out=stats[:, p * (QP // 512) + n, :],
                                in_=qfv[:, n, :],
                            )
                        pieces.append(qf)
                    qpieces.append(pieces)
                    statst.append(stats)
                return qpieces, statst

            def emit_norm_fold(statst):
                """bn_aggr + Newton rsqrt -> rstd y; scale Wg/Wqk by y and
                build the -mu*r projection biases via tiny matmuls."""
                mvs = []
                for cc in range(NCC):
                    mv = p_stats.tile([128, 2], f32, tag=f"mv{cc}", name="mv")
                    nc.vector.bn_aggr(out=mv, in_=statst[cc])
                    mvs.append(mv)
                s_ = p_stats.tile([128, 2], f32, tag="nt_s", name="s_")
                for cc in range(NCC):
                    nc.vector.tensor_scalar(
                        out=s_[:, cc:cc + 1], in0=mvs[cc][:, 1:2],
                        scalar1=1e-5, scalar2=None, op0=OP.add,
                    )
                t1i = p_stats.tile([128, 2], i32, tag="nt_t1", name="t1i")
                nc.vector.tensor_scalar(
                    out=t1i, in0=s_.bitcast(i32), scalar1=1, scalar2=None,
                    op0=OP.arith_shift_right,
                )
                y0i = p_stats.tile([128, 2], i32, tag="nt_y0", name="y0i")
                nc.vector.tensor_scalar(
                    out=y0i, in0=t1i, scalar1=-1, scalar2=0x5F3759DF,
                    op0=OP.mult, op1=OP.add,
                )
                y = y0i.bitcast(f32)
                for it in range(3):
                    aa = p_stats.tile([128, 2], f32, tag=f"nt_a{it}", name="aa")
                    nc.vector.tensor_mul(out=aa, in0=y, in1=y)
                    nc.vector.tensor_mul(out=aa, in0=aa, in1=s_)
                    nc.vector.tensor_scalar(
                        out=aa, in0=aa, scalar1=-0.5, scalar2=1.5,
                        op0=OP.mult, op1=OP.add,
                    )
                    yn = p_stats.tile([128, 2], f32, tag=f"nt_y{it}", name="yn")
                    nc.vector.tensor_mul(out=yn, in0=y, in1=aa)
                    y = yn
                # murn = -mu * rstd (bf16: matmul operand next to bf16 weights)
                murn = p_stats.tile([128, 2], bf16, tag="nt_mr", name="murn")
                for cc in range(NCC):
                    nc.vector.tensor_scalar(
                        out=murn[:, cc:cc + 1], in0=mvs[cc][:, 0:1],
                        scalar1=y[:, cc:cc + 1], scalar2=-1.0,
                        op0=OP.mult, op1=OP.mult,
                    )
                # scaled weights
                wg_s = []
                wqk_s = []
                for cc in range(NCC):
                    t_ = p_const.tile([128, HID], bf16, tag=f"wgs{cc}",
                                      name=f"wgs{cc}")
                    nc.vector.tensor_scalar(
                        out=t_, in0=wg_sb[cc], scalar1=y[:, cc:cc + 1],
                        scalar2=None, op0=OP.mult,
                    )
                    wg_s.append(t_)
                    t_ = p_const.tile([128, QK], bf16, tag=f"wqks{cc}",
                                      name=f"wqks{cc}")
                    nc.vector.tensor_scalar(
                        out=t_, in0=wqk_sb[cc], scalar1=y[:, cc:cc + 1],
                        scalar2=None, op0=OP.mult,
                    )
                    wqk_s.append(t_)
                # biases K[:, j] = sum_c murn[c] * W_s[c, block_j]
                psK = psA.tile([128, 8], f32, tag="psA", name="psK")
                for hc in range(NHC):
                    for cc in range(NCC):
                        nc.tensor.matmul(
                            psK[:, hc:hc + 1],
                            wg_s[cc][:, hc * 128:(hc + 1) * 128],
                            murn[:, cc:cc + 1],
                            start=(cc == 0), stop=(cc == NCC - 1),
                        )
                for cc in range(NCC):
                    nc.tensor.matmul(
                        psK[:, NHC:NHC + 1], wqk_s[cc][:, :],
                        murn[:, cc:cc + 1],
                        start=(cc == 0), stop=(cc == NCC - 1),
                    )
                biasK = p_stats.tile([128, 8], f32, tag="biasK", name="biasK")
                nc.vector.tensor_copy(out=biasK, in_=psK)
                return wg_s, wqk_s, biasK

            # ---------------- supertile pipeline ----------------
            st_state = {}

            def emit_vh(st):
                # v-side work — independent of the q stats, so it pre-runs
                # and keeps PE busy during the prologue. v is host-premasked.
                t0 = st * ST
                vb = []
                for cc in range(NCC):
                    vb_t = p_vstage.tile([128, ST], bf16, tag="vbf", name="vb_t")
                    nc.sync.dma_start(
                        out=vb_t, in_=v_d[cc * 128:(cc + 1) * 128, t0:t0 + ST]
                    )
                    vb.append(vb_t)
                vm = []
                for g in range(GPS):
                    pv = psA.tile([128, HID], f32, tag="psA", name="pv")
                    for cc in range(NCC):
                        nc.tensor.matmul(
                            pv[:, :],
                            vb[cc][:, g * G:(g + 1) * G],
                            wv_sb[cc][:, :],
                            start=(cc == 0), stop=(cc == NCC - 1),
                        )
                    vm_t = p_stx.tile([128, HID], bf16, tag="vm", name="vm_t",
                                      bufs=40)
                    nc.scalar.activation(out=vm_t, in_=pv, func=AF.Silu)
                    vm.append(vm_t)
                st_state[st] = dict(vm=vm)

            def qmov(qpieces, cc, st):
                p = st // (NQP)
                off = (st % NQP) * ST
                # piece p spans NST//NQP supertiles of 512 tokens
                return qpieces[cc][p][:, off:off + ST]

            def emit_qproj(st, qpieces, wg_s, wqk_s, biasK):
                # qkT = silu(Wqk_s^T q + Kq): [QK, ST]
                pq = psA.tile([128, ST], f32, tag="psA", name="pq")
                for cc in range(NCC):
                    nc.tensor.matmul(
                        pq[:, :], wqk_s[cc][:, :], qmov(qpieces, cc, st),
                        start=(cc == 0), stop=(cc == NCC - 1),
                    )
                qkT = p_st.tile([128, ST], bf16, tag="qkT", name="qkT")
                nc.scalar.activation(
                    out=qkT, in_=pq, func=AF.Silu, bias=biasK[:, NHC:NHC + 1]
                )
                # packed [qsA_g | qsB_g] per group for the merged sim/S matmul
                qsAB = p_st.tile([128, GPS * 2 * G], bf16, tag="qsAB",
                                 name="qsAB")
                qsABv = qsAB.rearrange("p (g two f) -> p g two f", two=2, f=G)
                qkTv = qkT.rearrange("p (g f) -> p g f", f=G)
                nc.vector.tensor_scalar(
                    out=qsABv[:, :, 0, :], in0=qkTv, scalar1=gA_sb,
                    scalar2=None, op0=OP.mult,
                )
                nc.vector.tensor_scalar(
                    out=qsABv[:, :, 1, :], in0=qkTv, scalar1=gB_sb,
                    scalar2=None, op0=OP.mult,
                )
                # gateT = silu(Wg_s^T q + Kg): 4 h-chunks [128h, ST]
                gate = []
                for hc in range(NHC):
                    pg = psA.tile([128, ST], f32, tag="psA", name="pg")
                    for cc in range(NCC):
                        nc.tensor.matmul(
                            pg[:, :],
                            wg_s[cc][:, hc * 128:(hc + 1) * 128],
                            qmov(qpieces, cc, st),
                            start=(cc == 0), stop=(cc == NCC - 1),
                        )
                    g_t = p_stx.tile([128, ST], bf16, tag="gate", name="g_t")
                    nc.scalar.activation(
                        out=g_t, in_=pg, func=AF.Silu,
                        bias=biasK[:, hc:hc + 1],
                    )
                    gate.append(g_t)
                st_state[st].update(qkT=qkT, qsAB=qsAB, gate=gate)

            def emit_attn_find(st, late):
                # ONE matmul per group produces [sim_g | S_g] packed in a
                # [128, 1024] psum; laplace runs on strided views.
                # R' = tanh(arg) + (2S+1) = 2*(laplace+S); the 0.5 is folded
                # into Wo on the host. Chain uses only tensor_tensor /
                # tensor_scalar ops (DVE 2x fast modes; stt has none).
                S = st_state[st]
                psAB = psAttn.tile([128, GPS * 2 * G], f32, tag="psAt",
                                   name="psAB")
                for g in range(GPS):
                    nc.tensor.matmul(
                        psAB[:, g * 2 * G:(g + 1) * 2 * G],
                        S["qkT"][:, g * G:(g + 1) * G],
                        S["qsAB"][:, g * 2 * G:(g + 1) * 2 * G],
                        start=True, stop=True,
                    )
                psv = psAB.rearrange("p (g two f) -> p g two f", two=2, f=G)
                # zzr = D1*sim + D2 (ACT reads PSUM, writes bf16 SBUF)
                zzr = p_lap.tile([128, ST], bf16, tag="zzr", name="zzr")
                nc.scalar.activation(
                    out=zzr, in_=psv[:, :, 0, :], func=AF.Identity,
                    bias=bias_d2, scale=D1_L,
                )
                # stl2 = 2*S + 1 (PSUM read; DVE early, ACT when vh is done)
                stl2 = p_lap.tile([128, ST], bf16, tag="stl", name="stl2")
                if late:
                    nc.scalar.activation(
                        out=stl2, in_=psv[:, :, 1, :], func=AF.Identity,
                        bias=bias_one, scale=2.0,
                    )
                else:
                    nc.vector.tensor_scalar(
                        out=stl2, in0=psv[:, :, 1, :], scalar1=2.0,
                        scalar2=1.0, op0=OP.mult, op1=OP.add,
                    )
                # w = zzr^2 ; u = w*zzr ; tt2 = u + zzr ; th = tanh(AB*tt2)
                w = p_lap.tile([128, ST], bf16, tag="w", name="w")
                nc.gpsimd.tensor_mul(out=w, in0=zzr, in1=zzr)
                u = p_lap.tile([128, ST], bf16, tag="u", name="u")
                nc.gpsimd.tensor_mul(out=u, in0=w, in1=zzr)
                tt2 = p_lap.tile([128, ST], bf16, tag="tt2", name="tt2")
                nc.gpsimd.tensor_add(out=tt2, in0=u, in1=zzr)
                th = p_lap.tile([128, ST], bf16, tag="th", name="th")
                nc.scalar.activation(out=th, in_=tt2, func=AF.Tanh,
                                     scale=AB_SCALE)
                R = p_carry.tile([128, ST], bf16, tag="R", name="R")
                nc.gpsimd.tensor_add(out=R, in0=th, in1=stl2)
                S["R"] = R

            def emit_attn_apply(st, late):
                t0 = st * ST
                S = st_state[st]
                # z^T[ec] = sum_g vm_g[:,ec]^T @ R_g; ec-major so each pz bank
                # is consumed right after its 4 MMs
                z = []
                for ec in range(NHC):
                    pz = psZ.tile([128, ST], f32, tag="psZ", name=f"pz{ec}")
                    for g in range(GPS):
                        sl = slice(g * G, (g + 1) * G)
                        nc.tensor.matmul(
                            pz[:, sl],
                            S["vm"][g][:, ec * 128:(ec + 1) * 128],
                            S["R"][:, sl],
                            start=True, stop=True,
                        )
                    z_t = p_out.tile([128, ST], bf16, tag=f"z{ec}", name=f"z{ec}")
                    nc.vector.tensor_mul(out=z_t, in0=pz, in1=S["gate"][ec])
                    z.append(z_t)
                # out^T = Wo'^T z (Wo pre-halved on host); the m1 token mask is
                # applied on the host, so this is a pure PSUM->SBUF f32 copy —
                # DVE early, ACT once vh silus are done.
                for oc in range(NOC):
                    po = psA.tile([128, ST], f32, tag="psA", name="po")
                    for hc in range(NHC):
                        nc.tensor.matmul(
                            po[:, :],
                            wo_sb[hc][:, oc * 128:(oc + 1) * 128],
                            z[hc][:, :],
                            start=(hc == 0), stop=(hc == NHC - 1),
                        )
                    ot = p_out.tile([128, ST], f32, tag="oc", name="ot")
                    if late and oc == 0:
                        nc.scalar.activation(out=ot, in_=po, func=AF.Identity)
                    else:
                        nc.vector.tensor_copy(out=ot, in_=po)
                    nc.sync.dma_start(
                        out=out_d[oc * 128:(oc + 1) * 128, t0:t0 + ST], in_=ot
                    )
                del st_state[st]

            PRE_K = 6
            # vh emission stops at iteration NST-PRE_K; later iterations have
            # spare ACT capacity, so PSUM->SBUF copies flex DVE -> ACT there.
            LATE = NST - PRE_K
            qpieces, statst = emit_q_stats()
            for st in range(PRE_K):
                emit_vh(st)
            wg_s, wqk_s, biasK = emit_norm_fold(statst)
            # apply(st-1) is emitted BEFORE find(st): engine queues are FIFO
            # in emission order, so this keeps the z-gate multiplies (which
            # free the psZ banks the PE z-matmuls rotate through) ahead of
            # find's laplace chain on the DVE queue.
            for st in range(NST):
                emit_qproj(st, qpieces, wg_s, wqk_s, biasK)
                if st >= 1:
                    emit_attn_apply(st - 1, late=(st - 1 >= LATE - 1))
                emit_attn_find(st, late=(st >= LATE))
                if st + PRE_K < NST:
                    emit_vh(st + PRE_K)
            emit_attn_apply(NST - 1, late=True)

    nc.compile()
    return nc


def _get_program():
    global _PROG
    if _PROG is None:
        _PROG = _build_program()
    return _PROG


def _host_prep(inputs):
    """Build per-core input maps. Returns (in_maps, None) for the fast path
    or (None, reason) when the fast path's preconditions fail."""
    bf = ml_dtypes.bfloat16
    q = np.ascontiguousarray(np.asarray(inputs["q"], dtype=np.float32).astype(bf))
    masks = np.asarray(inputs["masks"], dtype=np.float32)
    for name in ("bg", "bv", "bqk", "bo", "beta"):
        if np.any(np.asarray(inputs[name]) != 0.0):
            return None, f"nonzero {name}"

    gamma = np.asarray(inputs["gamma"], dtype=np.float32)
    gA = (gamma[0] * gamma[2] / G).reshape(QK, 1).astype(np.float32)
    gB = (gamma[1] * gamma[3] / T).reshape(QK, 1).astype(np.float32)
    wg = np.asarray(inputs["Wg"], dtype=np.float32).astype(bf)
    wv = np.asarray(inputs["Wv"], dtype=np.float32).astype(bf)
    wqk = np.asarray(inputs["Wqk"], dtype=np.float32).astype(bf)
    # device computes R' = 2*(laplace+S); the 0.5 is folded into Wo
    wo = (np.asarray(inputs["Wo"], dtype=np.float32) * 0.5).astype(bf)

    # gen_key_padding_mask: all-zero mask batches are reset to ones
    m1 = np.where(masks.sum(axis=(1, 2), keepdims=True) == 0.0, 1.0, masks)
    m1 = m1[:, 0, :].astype(np.float32)          # [B, T]
    m0 = 1.0 - m1                                 # 1 where mask==0
    # premask v: absorbs the attn j-mask and the lin_k n-mask
    vm_host = (np.asarray(inputs["v"], dtype=np.float32)
               * m0[:, None, :]).astype(bf)

    in_maps = []
    for b in range(B):
        in_maps.append({
            "q": q[b],
            "v": np.ascontiguousarray(vm_host[b]),
            "wg": wg, "wv": wv, "wqk": wqk, "wo": wo,
            "gA": gA, "gB": gB,
        })
    return in_maps, m1


def _numpy_fallback(inputs):
    """Exact-semantics fp32 fallback for inputs outside the fast path
    (nonzero biases/beta). Mirrors the reference in numpy."""
    from scipy.special import erf

    def silu(x):
        return x / (1.0 + np.exp(-x))

    q = np.asarray(inputs["q"], np.float32)
    v = np.asarray(inputs["v"], np.float32)
    masks = np.asarray(inputs["masks"], np.float32)
    Wg, bg = np.asarray(inputs["Wg"], np.float32), np.asarray(inputs["bg"], np.float32)
    Wv, bv = np.asarray(inputs["Wv"], np.float32), np.asarray(inputs["bv"], np.float32)
    Wqk, bqk = np.asarray(inputs["Wqk"], np.float32), np.asarray(inputs["bqk"], np.float32)
    gamma, beta = np.asarray(inputs["gamma"], np.float32), np.asarray(inputs["beta"], np.float32)
    Wo, bo = np.asarray(inputs["Wo"], np.float32), np.asarray(inputs["bo"], np.float32)

    all_zero = masks.sum(axis=(1, 2)) == 0.0
    masks = np.where(all_zero[:, None, None], 1.0, masks)
    kpm = masks[:, 0, :] == 0.0
    mu = q.mean(-1, keepdims=True)
    var = q.var(-1, keepdims=True)
    qn = (q - mu) / np.sqrt(var + 1e-5)
    x = qn.transpose(0, 2, 1)
    vt = v.transpose(0, 2, 1)
    gate = silu(x @ Wg + bg)
    vh = silu(vt @ Wv + bv)
    qk = silu(x @ Wqk + bqk)
    qk4 = qk[..., None, :] * gamma + beta
    quad_q, lin_q, quad_k, lin_k = (qk4[..., i, :] for i in range(4))
    lin_k = np.where(kpm[..., None], lin_k, 0.0)
    ng = T // G
    grp = lambda t: t.reshape(B, ng, G, t.shape[-1])
    qq, lq, qkk, lk, vg = map(grp, (quad_q, lin_q, quad_k, lin_k, vh))
    kpm_g = kpm.reshape(B, ng, 1, G)
    sim = np.einsum("bgid,bgjd->bgij", qq, qkk) / G
    attn = (1.0 + erf((sim - MU_L) / (STD_L * math.sqrt(2.0)))) * 0.5
    attn = np.where(kpm_g, attn, 0.0)
    quad_out = np.einsum("bgij,bgje->bgie", attn, vg)
    lin_kv = np.einsum("bgnd,bgne->bgde", lk, vg) / T
    lin_out = np.einsum("bgnd,bgde->bgne", lq, lin_kv)
    out = gate * (quad_out + lin_out).reshape(B, T, HID)
    out = (out @ Wo + bo).transpose(0, 2, 1)
    return (out * masks).astype(np.float32)


def kernel(**inputs):
    in_maps, m1 = _host_prep(inputs)
    if in_maps is None:
        return _numpy_fallback(inputs)

    from concourse.bass_utils import run_bass_kernel_spmd

    nc = _get_program()
    core_ids = list(range(8))
    res = run_bass_kernel_spmd(nc, in_maps, core_ids)
    out = np.empty((B, C, T), np.float32)
    for b in range(B):
        # final token mask applied host-side (elementwise, exact)
        out[b] = np.asarray(res.results[b]["out"], dtype=np.float32) \
            * m1[b][None, :]
    return out


if __name__ == "__main__":
    rng = np.random.default_rng(0)
    ins = {
        "q": rng.standard_normal((B, C, T), dtype=np.float32),
        "k": rng.standard_normal((B, C, T), dtype=np.float32),
        "v": rng.standard_normal((B, C, T), dtype=np.float32),
        "masks": rng.integers(0, 2, (B, 1, T)).astype(np.float32),
        "Wg": (rng.standard_normal((C, HID)) * 0.02).astype(np.float32),
        "bg": np.zeros(HID, np.float32),
        "Wv": (rng.standard_normal((C, HID)) * 0.02).astype(np.float32),
        "bv": np.zeros(HID, np.float32),
        "Wqk": (rng.standard_normal((C, QK)) * 0.02).astype(np.float32),
        "bqk": np.zeros(QK, np.float32),
        "gamma": (1 + rng.standard_normal((4, QK)) * 0.02).astype(np.float32),
        "beta": np.zeros((4, QK), np.float32),
        "Wo": (rng.standard_normal((HID, C)) * 0.02).astype(np.float32),
        "bo": np.zeros(C, np.float32),
    }
    got = kernel(**ins)
    exp = _numpy_fallback(ins)
    err = np.abs(got - exp).max() / np.abs(exp).max()
    print("absmax-rel err vs numpy:", err)


# revision 25
# speedup vs baseline: 1.1468x; 1.1468x over previous
"""Trainium2 Bass kernel for nn_MixedChunkAttentionLayer.

Sharding: pure data-parallel over batch — B=8 batches onto 8 NeuronCores,
one batch per core, zero cross-core communication.

Per-core pipeline (batch b, C=256, T=8192, G=128, QK=128, HID=512):
  - InstanceNorm is folded into the projections: bn_stats/bn_aggr + Newton
    rsqrt give rstd r and mean mu per channel; the kernel scales
    Wg/Wqk by r on-device (Wg_s = r*Wg) and computes per-feature bias
    K = (-mu*r)^T W_s via tiny matmuls, so the projections consume the
    RAW bf16 q pieces as the moving operand and the silu ACT applies the
    bias: gate = silu(Wg_s^T q + K). No qn materialization pass.
  - v arrives host-premasked (v*m0, m0=1 where mask==0) which absorbs both
    the quadratic attn j-mask and the linear lin_k n-mask, so vm needs no
    per-group scale and the OffsetScale gammas fold into qsA/qsB:
      qsA = qkT*(g0*g2/G), qsB = qkT*(g1*g3/T)
  - The linear branch collapses into the quadratic one (R = laplace(sim)+S);
    sim and S for each group are produced by ONE matmul with the moving
    operand [qsA_g | qsB_g] packed [128,256], psum [128,1024] per supertile.
  - laplace chain engineered for engine balance: the two PSUM reads run on
    ACT (zzr = D1*sim+D2 -> bf16) and DVE (stl = S+0.5 -> bf16); the rest
    are cheap all-SBUF bf16 DVE ops (w = zzr^2, tt2 = (1+w)*zzr,
    R = 0.5*tanh(AB*tt2)+stl) eligible for the DVE 2x/4x fast modes.
  - z^T[ec] = vm_g[:,ec]^T @ R_g; z = z^T * gateT; out^T = Wo^T z, * m1,
    stored bf16 (host upcasts to f32).

laplace_attn(x) = Phi((x-mu)/sigma) is evaluated as
  0.5*(1 + tanh(zz*(a + b*zz^2))), zz=(x-mu)/sigma
(max abs err 1.8e-4) so every ACT function used (Silu/Tanh/Identity)
lives in the single `silu_and_others` table set — no table reloads.
"""

import math
import sys

if "/opt/trn_rl_repo" not in sys.path:
    sys.path.insert(0, "/opt/trn_rl_repo")

import numpy as np
import ml_dtypes

B, C, T = 8, 256, 8192
G = 128
QK = 128
HID = 512
NG = T // G          # 64 groups
ST = 512             # supertile token count
NST = T // ST        # 16 supertiles
GPS = ST // G        # 4 groups per supertile
NCC = C // 128       # 2 contraction chunks
NHC = HID // 128     # 4 HID chunks
NOC = C // 128       # 2 output-channel chunks

MU_L = math.sqrt(0.5)
STD_L = math.sqrt(0.25 * math.pi)
S1_L = 1.0 / STD_L           # zz = S1*x + C1
C1_L = -MU_L / STD_L
A_C = math.sqrt(2.0 / math.pi)
B_C = A_C * 0.044715
# zzr = D1*x + D2 = sqrt(B/A)*zz ; w = zzr^2 = (B/A)*zz^2
# tanh arg = A*zz*(1+w) = AB_SCALE * (1+w)*zzr
_RBA = math.sqrt(B_C / A_C)
D1_L = S1_L * _RBA
D2_L = C1_L * _RBA
AB_SCALE = A_C / _RBA

_PROG = None  # cached — program is input-independent


def _build_program():
    import concourse.bass as bass
    import concourse.tile as tile
    from concourse import bacc, mybir

    f32 = mybir.dt.float32
    bf16 = mybir.dt.bfloat16
    i32 = mybir.dt.int32
    AF = mybir.ActivationFunctionType
    OP = mybir.AluOpType

    nc = bacc.Bacc("TRN2", target_bir_lowering=False, debug=False, num_devices=8)

    q_d = nc.dram_tensor("q", [C, T], bf16, kind="ExternalInput")
    v_d = nc.dram_tensor("v", [C, T], bf16, kind="ExternalInput")
    wg_d = nc.dram_tensor("wg", [C, HID], bf16, kind="ExternalInput")
    wv_d = nc.dram_tensor("wv", [C, HID], bf16, kind="ExternalInput")
    wqk_d = nc.dram_tensor("wqk", [C, QK], bf16, kind="ExternalInput")
    wo_d = nc.dram_tensor("wo", [HID, C], bf16, kind="ExternalInput")
    gA_d = nc.dram_tensor("gA", [QK, 1], f32, kind="ExternalInput")
    gB_d = nc.dram_tensor("gB", [QK, 1], f32, kind="ExternalInput")
    out_d = nc.dram_tensor("out", [C, T], f32, kind="ExternalOutput")

    with tile.TileContext(nc) as tc:
        with (
            tc.tile_pool(name="const", bufs=1) as p_const,
            tc.tile_pool(name="qstage", bufs=1) as p_qstage,
            tc.tile_pool(name="stats", bufs=2) as p_stats,
            tc.tile_pool(name="vstage", bufs=16) as p_vstage,
            tc.tile_pool(name="stw", bufs=2) as p_st,          # within-supertile
            tc.tile_pool(name="stx", bufs=8) as p_stx,         # live 2 supertiles
            tc.tile_pool(name="lap", bufs=2) as p_lap,         # laplace temps
            tc.tile_pool(name="carry", bufs=2) as p_carry,     # R across phases
            tc.tile_pool(name="outp", bufs=2) as p_out,
            tc.tile_pool(name="psA", bufs=4, space="PSUM") as psA,
            tc.tile_pool(name="psAttn", bufs=1, space="PSUM") as psAttn,
            tc.tile_pool(name="psZ", bufs=2, space="PSUM") as psZ,
        ):
            # ---------------- constants ----------------
            # weight loads go on the gpsimd HWDGE queue so the sync queue is
            # free for the v supertile loads the PE prologue depends on;
            # wv first — the vh prelude needs it.
            wg_sb = []
            wv_sb = []
            wqk_sb = []
            for cc in range(NCC):
                t_ = p_const.tile([128, HID], bf16, tag=f"wv{cc}", name=f"wv{cc}")
                nc.gpsimd.dma_start(out=t_, in_=wv_d[cc * 128:(cc + 1) * 128, :])
                wv_sb.append(t_)
            gA_sb = p_const.tile([QK, 1], f32, tag="gA")
            nc.gpsimd.dma_start(out=gA_sb, in_=gA_d[:, :])
            gB_sb = p_const.tile([QK, 1], f32, tag="gB")
            nc.gpsimd.dma_start(out=gB_sb, in_=gB_d[:, :])
            wo_sb = []

            def emit_late_weights():
                # emitted after the first vh v-loads so the gpsimd DMA ring
                # serves the PE prologue's v tiles first
                for cc in range(NCC):
                    t_ = p_const.tile([128, HID], bf16, tag=f"wg{cc}",
                                      name=f"wg{cc}")
                    nc.gpsimd.dma_start(
                        out=t_, in_=wg_d[cc * 128:(cc + 1) * 128, :])
                    wg_sb.append(t_)
                    t_ = p_const.tile([128, QK], bf16, tag=f"wqk{cc}",
                                      name=f"wqk{cc}")
                    nc.gpsimd.dma_start(
                        out=t_, in_=wqk_d[cc * 128:(cc + 1) * 128, :])
                    wqk_sb.append(t_)
                for hc in range(NHC):
                    t_ = p_const.tile([128, C], bf16, tag=f"wo{hc}",
                                      name=f"wo{hc}")
                    nc.gpsimd.dma_start(
                        out=t_, in_=wo_d[hc * 128:(hc + 1) * 128, :])
                    wo_sb.append(t_)

            bias_d2 = p_const.tile([128, 1], f32, tag="bias_d2")
            nc.vector.memset(bias_d2, D2_L)
            bias_one = p_const.tile([128, 1], f32, tag="bias_one")
            nc.vector.memset(bias_one, 1.0)

            # ---------------- q staging + stats ----------------
            # q streamed in [128, QP] pieces; the SAME pieces stay resident as
            # the projections' moving operand (instance norm is folded into
            # the weights), so q is read from HBM exactly once.
            NQP = 4
            QP = T // NQP

            def emit_q_stats():
                qpieces = []
                statst = []
                for cc in range(NCC):
                    pieces = []
                    stats = p_stats.tile([128, T // 512, 6], f32,
                                         tag=f"bnstats{cc}", name="stats")
                    for p in range(NQP):
                        qf = p_qstage.tile([128, QP], bf16, tag=f"qf{cc}{p}",
                                           name="qf", bufs=1)
                        # q alternates between the scalar and sync HWDGE
                        # queues for ~2x the issue share — q gates the stats
                        # and therefore the whole projection pipeline.
                        eng = nc.scalar if (cc * NQP + p) % 2 == 0 else nc.sync
                        eng.dma_start(
                            out=qf,
                            in_=q_d[cc * 128:(cc + 1) * 128,
                                    p * QP:(p + 1) * QP],
                        )
                        qfv = qf.rearrange("p (n f) -> p n f", f=512)
                        for n in range(QP // 512):
                            nc.vector.bn_stats(
                                out=stats[:, p * (QP // 512) + n, :],
                                in_=qfv[:, n, :],
                            )
                        pieces.append(qf)
                    qpieces.append(pieces)
                    statst.append(stats)
                return qpieces, statst

            def emit_norm_fold(statst):
                """bn_aggr + Newton rsqrt -> rstd y; scale Wg/Wqk by y and
                build the -mu*r projection biases via tiny matmuls."""
                mvs = []
                for cc in range(NCC):
                    mv = p_stats.tile([128, 2], f32, tag=f"mv{cc}", name="mv")
                    nc.vector.bn_aggr(out=mv, in_=statst[cc])
                    mvs.append(mv)
                s_ = p_stats.tile([128, 2], f32, tag="nt_s", name="s_")
                for cc in range(NCC):
                    nc.vector.tensor_scalar(
                        out=s_[:, cc:cc + 1], in0=mvs[cc][:, 1:2],
                        scalar1=1e-5, scalar2=None, op0=OP.add,
                    )
                t1i = p_stats.tile([128, 2], i32, tag="nt_t1", name="t1i")
                nc.vector.tensor_scalar(
                    out=t1i, in0=s_.bitcast(i32), scalar1=1, scalar2=None,
                    op0=OP.arith_shift_right,
                )
                y0i = p_stats.tile([128, 2], i32, tag="nt_y0", name="y0i")
                nc.vector.tensor_scalar(
                    out=y0i, in0=t1i, scalar1=-1, scalar2=0x5F3759DF,
                    op0=OP.mult, op1=OP.add,
                )
                y = y0i.bitcast(f32)
                for it in range(3):
                    aa = p_stats.tile([128, 2], f32, tag=f"nt_a{it}", name="aa")
                    nc.vector.tensor_mul(out=aa, in0=y, in1=y)
                    nc.vector.tensor_mul(out=aa, in0=aa, in1=s_)
                    nc.vector.tensor_scalar(
                        out=aa, in0=aa, scalar1=-0.5, scalar2=1.5,
                        op0=OP.mult, op1=OP.add,
                    )
                    yn = p_stats.tile([128, 2], f32, tag=f"nt_y{it}", name="yn")
                    nc.vector.tensor_mul(out=yn, in0=y, in1=aa)
                    y = yn
                # murn = -mu * rstd (bf16: matmul operand next to bf16 weights)
                murn = p_stats.tile([128, 2], bf16, tag="nt_mr", name="murn")
                for cc in range(NCC):
                    nc.vector.tensor_scalar(
                        out=murn[:, cc:cc + 1], in0=mvs[cc][:, 0:1],
                        scalar1=y[:, cc:cc + 1], scalar2=-1.0,
                        op0=OP.mult, op1=OP.mult,
                    )
                # scaled weights
                wg_s = []
                wqk_s = []
                for cc in range(NCC):
                    t_ = p_const.tile([128, HID], bf16, tag=f"wgs{cc}",
                                      name=f"wgs{cc}")
                    nc.vector.tensor_scalar(
                        out=t_, in0=wg_sb[cc], scalar1=y[:, cc:cc + 1],
                        scalar2=None, op0=OP.mult,
                    )
                    wg_s.append(t_)
                    t_ = p_const.tile([128, QK], bf16, tag=f"wqks{cc}",
                                      name=f"wqks{cc}")
                    nc.vector.tensor_scalar(
                        out=t_, in0=wqk_sb[cc], scalar1=y[:, cc:cc + 1],
                        scalar2=None, op0=OP.mult,
                    )
                    wqk_s.append(t_)
                # biases K[:, j] = sum_c murn[c] * W_s[c, block_j]
                psK = psA.tile([128, 8], f32, tag="psA", name="psK")
                for hc in range(NHC):
                    for cc in range(NCC):
                        nc.tensor.matmul(
                            psK[:, hc:hc + 1],
                            wg_s[cc][:, hc * 128:(hc + 1) * 128],
                            murn[:, cc:cc + 1],
                            start=(cc == 0), stop=(cc == NCC - 1),
                        )
                for cc in range(NCC):
                    nc.tensor.matmul(
                        psK[:, NHC:NHC + 1], wqk_s[cc][:, :],
                        murn[:, cc:cc + 1],
                        start=(cc == 0), stop=(cc == NCC - 1),
                    )
                biasK = p_stats.tile([128, 8], f32, tag="biasK", name="biasK")
                nc.vector.tensor_copy(out=biasK, in_=psK)
                return wg_s, wqk_s, biasK

            # ---------------- supertile pipeline ----------------
            st_state = {}

            def emit_vh(st):
                # v-side work — independent of the q stats, so it pre-runs
                # and keeps PE busy during the prologue. v is host-premasked.
                t0 = st * ST
                vb = []
                for cc in range(NCC):
                    vb_t = p_vstage.tile([128, ST], bf16, tag="vbf", name="vb_t")
                    nc.gpsimd.dma_start(
                        out=vb_t, in_=v_d[cc * 128:(cc + 1) * 128, t0:t0 + ST]
                    )
                    vb.append(vb_t)
                vm = []
                for g in range(GPS):
                    pv = psA.tile([128, HID], f32, tag="psA", name="pv")
                    for cc in range(NCC):
                        nc.tensor.matmul(
                            pv[:, :],
                            vb[cc][:, g * G:(g + 1) * G],
                            wv_sb[cc][:, :],
                            start=(cc == 0), stop=(cc == NCC - 1),
                        )
                    vm_t = p_stx.tile([128, HID], bf16, tag="vm", name="vm_t",
                                      bufs=40)
                    nc.scalar.activation(out=vm_t, in_=pv, func=AF.Silu)
                    vm.append(vm_t)
                st_state[st] = dict(vm=vm)

            def qmov(qpieces, cc, st):
                p = st // (NQP)
                off = (st % NQP) * ST
                # piece p spans NST//NQP supertiles of 512 tokens
                return qpieces[cc][p][:, off:off + ST]

            def emit_qproj(st, qpieces, wg_s, wqk_s, biasK):
                # qkT = silu(Wqk_s^T q + Kq): [QK, ST]
                pq = psA.tile([128, ST], f32, tag="psA", name="pq")
                for cc in range(NCC):
                    nc.tensor.matmul(
                        pq[:, :], wqk_s[cc][:, :], qmov(qpieces, cc, st),
                        start=(cc == 0), stop=(cc == NCC - 1),
                    )
                qkT = p_st.tile([128, ST], bf16, tag="qkT", name="qkT")
                nc.scalar.activation(
                    out=qkT, in_=pq, func=AF.Silu, bias=biasK[:, NHC:NHC + 1]
                )
                # packed [qsA_g | qsB_g] per group for the merged sim/S matmul
                qsAB = p_st.tile([128, GPS * 2 * G], bf16, tag="qsAB",
                                 name="qsAB")
                qsABv = qsAB.rearrange("p (g two f) -> p g two f", two=2, f=G)
                qkTv = qkT.rearrange("p (g f) -> p g f", f=G)
                nc.vector.tensor_scalar(
                    out=qsABv[:, :, 0, :], in0=qkTv, scalar1=gA_sb,
                    scalar2=None, op0=OP.mult,
                )
                nc.vector.tensor_scalar(
                    out=qsABv[:, :, 1, :], in0=qkTv, scalar1=gB_sb,
                    scalar2=None, op0=OP.mult,
                )
                # gateT = silu(Wg_s^T q + Kg): 4 h-chunks [128h, ST]
                gate = []
                for hc in range(NHC):
                    pg = psA.tile([128, ST], f32, tag="psA", name="pg")
                    for cc in range(NCC):
                        nc.tensor.matmul(
                            pg[:, :],
                            wg_s[cc][:, hc * 128:(hc + 1) * 128],
                            qmov(qpieces, cc, st),
                            start=(cc == 0), stop=(cc == NCC - 1),
                        )
                    g_t = p_stx.tile([128, ST], bf16, tag="gate", name="g_t")
                    nc.scalar.activation(
                        out=g_t, in_=pg, func=AF.Silu,
                        bias=biasK[:, hc:hc + 1],
                    )
                    gate.append(g_t)
                st_state[st].update(qkT=qkT, qsAB=qsAB, gate=gate)

            def emit_attn_find(st, late):
                # ONE matmul per group produces [sim_g | S_g] packed in a
                # [128, 1024] psum; laplace runs on strided views.
                # R' = tanh(arg) + (2S+1) = 2*(laplace+S); the 0.5 is folded
                # into Wo on the host. Chain uses only tensor_tensor /
                # tensor_scalar ops (DVE 2x fast modes; stt has none).
                S = st_state[st]
                psAB = psAttn.tile([128, GPS * 2 * G], f32, tag="psAt",
                                   name="psAB")
                for g in range(GPS):
                    nc.tensor.matmul(
                        psAB[:, g * 2 * G:(g + 1) * 2 * G],
                        S["qkT"][:, g * G:(g + 1) * G],
                        S["qsAB"][:, g * 2 * G:(g + 1) * 2 * G],
                        start=True, stop=True,
                    )
                psv = psAB.rearrange("p (g two f) -> p g two f", two=2, f=G)
                # zzr = D1*sim + D2 (ACT reads PSUM, writes bf16 SBUF)
                zzr = p_lap.tile([128, ST], bf16, tag="zzr", name="zzr")
                nc.scalar.activation(
                    out=zzr, in_=psv[:, :, 0, :], func=AF.Identity,
                    bias=bias_d2, scale=D1_L,
                )
                # stl2 = 2*S + 1 (PSUM read; DVE early, ACT when vh is done)
                stl2 = p_lap.tile([128, ST], bf16, tag="stl", name="stl2")
                if late:
                    nc.scalar.activation(
                        out=stl2, in_=psv[:, :, 1, :], func=AF.Identity,
                        bias=bias_one, scale=2.0,
                    )
                else:
                    nc.vector.tensor_scalar(
                        out=stl2, in0=psv[:, :, 1, :], scalar1=2.0,
                        scalar2=1.0, op0=OP.mult, op1=OP.add,
                    )
                # w = zzr^2 ; u = w*zzr ; tt2 = u + zzr ; th = tanh(AB*tt2)
                w = p_lap.tile([128, ST], bf16, tag="w", name="w")
                nc.vector.tensor_mul(out=w, in0=zzr, in1=zzr)
                u = p_lap.tile([128, ST], bf16, tag="u", name="u")
                nc.vector.tensor_mul(out=u, in0=w, in1=zzr)
                tt2 = p_lap.tile([128, ST], bf16, tag="tt2", name="tt2")
                nc.vector.tensor_add(out=tt2, in0=u, in1=zzr)
                th = p_lap.tile([128, ST], bf16, tag="th", name="th")
                nc.scalar.activation(out=th, in_=tt2, func=AF.Tanh,
                                     scale=AB_SCALE)
                R = p_carry.tile([128, ST], bf16, tag="R", name="R")
                nc.vector.tensor_add(out=R, in0=th, in1=stl2)
                S["R"] = R

            def emit_attn_apply_z(st, ecs):
                # z^T[ec] = sum_g vm_g[:,ec]^T @ R_g; ec-major so each pz bank
                # is consumed right after its 4 MMs
                S = st_state[st]
                z = S.setdefault("z", [])
                for ec in ecs:
                    pz = psZ.tile([128, ST], f32, tag="psZ", name=f"pz{ec}")
                    for g in range(GPS):
                        sl = slice(g * G, (g + 1) * G)
                        nc.tensor.matmul(
                            pz[:, sl],
                            S["vm"][g][:, ec * 128:(ec + 1) * 128],
                            S["R"][:, sl],
                            start=True, stop=True,
                        )
                    z_t = p_out.tile([128, ST], bf16, tag=f"z{ec}", name=f"z{ec}")
                    nc.vector.tensor_mul(out=z_t, in0=pz, in1=S["gate"][ec])
                    z.append(z_t)

            def emit_attn_out(st, late):
                t0 = st * ST
                S = st_state[st]
                z = S["z"]
                # out^T = Wo'^T z (Wo pre-halved on host); the m1 token mask is
                # applied on the host, so this is a pure PSUM->SBUF f32 copy —
                # DVE early, ACT once vh silus are done.
                for oc in range(NOC):
                    po = psA.tile([128, ST], f32, tag="psA", name="po")
                    for hc in range(NHC):
                        nc.tensor.matmul(
                            po[:, :],
                            wo_sb[hc][:, oc * 128:(oc + 1) * 128],
                            z[hc][:, :],
                            start=(hc == 0), stop=(hc == NHC - 1),
                        )
                    ot = p_out.tile([128, ST], f32, tag="oc", name="ot")
                    if late and oc == 0:
                        nc.scalar.activation(out=ot, in_=po, func=AF.Identity)
                    else:
                        nc.vector.tensor_copy(out=ot, in_=po)
                    nc.sync.dma_start(
                        out=out_d[oc * 128:(oc + 1) * 128, t0:t0 + ST], in_=ot
                    )
                del st_state[st]

            PRE_K = 6
            # vh emission stops at iteration NST-PRE_K; later iterations have
            # spare ACT capacity, so PSUM->SBUF copies flex DVE -> ACT there.
            LATE = NST - PRE_K
            qpieces, statst = emit_q_stats()
            for st in range(3):
                emit_vh(st)
            emit_late_weights()
            for st in range(3, PRE_K):
                emit_vh(st)
            wg_s, wqk_s, biasK = emit_norm_fold(statst)
            # apply(st-1) is emitted BEFORE find(st): engine queues are FIFO
            # in emission order, so this keeps the z-gate multiplies (which
            # free the psZ banks the PE z-matmuls rotate through) ahead of
            # find's laplace chain on the DVE queue. vh sits between the two
            # apply halves so PE has independent matmuls during the psZ
            # bank-recycle wait.
            for st in range(NST):
                emit_qproj(st, qpieces, wg_s, wqk_s, biasK)
                if st >= 1:
                    emit_attn_apply_z(st - 1, ecs=[0, 1])
                if st + PRE_K < NST:
                    emit_vh(st + PRE_K)
                if st >= 1:
                    emit_attn_apply_z(st - 1, ecs=[2, 3])
                    emit_attn_out(st - 1, late=(st - 1 >= LATE - 1))
                emit_attn_find(st, late=(st >= LATE))
            emit_attn_apply_z(NST - 1, ecs=[0, 1, 2, 3])
            emit_attn_out(NST - 1, late=True)

    nc.compile()
    return nc


def _get_program():
    global _PROG
    if _PROG is None:
        _PROG = _build_program()
    return _PROG


def _host_prep(inputs):
    """Build per-core input maps. Returns (in_maps, None) for the fast path
    or (None, reason) when the fast path's preconditions fail."""
    bf = ml_dtypes.bfloat16
    q = np.ascontiguousarray(np.asarray(inputs["q"], dtype=np.float32).astype(bf))
    masks = np.asarray(inputs["masks"], dtype=np.float32)
    for name in ("bg", "bv", "bqk", "bo", "beta"):
        if np.any(np.asarray(inputs[name]) != 0.0):
            return None, f"nonzero {name}"

    gamma = np.asarray(inputs["gamma"], dtype=np.float32)
    gA = (gamma[0] * gamma[2] / G).reshape(QK, 1).astype(np.float32)
    gB = (gamma[1] * gamma[3] / T).reshape(QK, 1).astype(np.float32)
    wg = np.asarray(inputs["Wg"], dtype=np.float32).astype(bf)
    wv = np.asarray(inputs["Wv"], dtype=np.float32).astype(bf)
    wqk = np.asarray(inputs["Wqk"], dtype=np.float32).astype(bf)
    # device computes R' = 2*(laplace+S); the 0.5 is folded into Wo
    wo = (np.asarray(inputs["Wo"], dtype=np.float32) * 0.5).astype(bf)

    # gen_key_padding_mask: all-zero mask batches are reset to ones
    m1 = np.where(masks.sum(axis=(1, 2), keepdims=True) == 0.0, 1.0, masks)
    m1 = m1[:, 0, :].astype(np.float32)          # [B, T]
    m0 = 1.0 - m1                                 # 1 where mask==0
    # premask v: absorbs the attn j-mask and the lin_k n-mask
    vm_host = (np.asarray(inputs["v"], dtype=np.float32)
               * m0[:, None, :]).astype(bf)

    in_maps = []
    for b in range(B):
        in_maps.append({
            "q": q[b],
            "v": np.ascontiguousarray(vm_host[b]),
            "wg": wg, "wv": wv, "wqk": wqk, "wo": wo,
            "gA": gA, "gB": gB,
        })
    return in_maps, m1


def _numpy_fallback(inputs):
    """Exact-semantics fp32 fallback for inputs outside the fast path
    (nonzero biases/beta). Mirrors the reference in numpy."""
    from scipy.special import erf

    def silu(x):
        return x / (1.0 + np.exp(-x))

    q = np.asarray(inputs["q"], np.float32)
    v = np.asarray(inputs["v"], np.float32)
    masks = np.asarray(inputs["masks"], np.float32)
    Wg, bg = np.asarray(inputs["Wg"], np.float32), np.asarray(inputs["bg"], np.float32)
    Wv, bv = np.asarray(inputs["Wv"], np.float32), np.asarray(inputs["bv"], np.float32)
    Wqk, bqk = np.asarray(inputs["Wqk"], np.float32), np.asarray(inputs["bqk"], np.float32)
    gamma, beta = np.asarray(inputs["gamma"], np.float32), np.asarray(inputs["beta"], np.float32)
    Wo, bo = np.asarray(inputs["Wo"], np.float32), np.asarray(inputs["bo"], np.float32)

    all_zero = masks.sum(axis=(1, 2)) == 0.0
    masks = np.where(all_zero[:, None, None], 1.0, masks)
    kpm = masks[:, 0, :] == 0.0
    mu = q.mean(-1, keepdims=True)
    var = q.var(-1, keepdims=True)
    qn = (q - mu) / np.sqrt(var + 1e-5)
    x = qn.transpose(0, 2, 1)
    vt = v.transpose(0, 2, 1)
    gate = silu(x @ Wg + bg)
    vh = silu(vt @ Wv + bv)
    qk = silu(x @ Wqk + bqk)
    qk4 = qk[..., None, :] * gamma + beta
    quad_q, lin_q, quad_k, lin_k = (qk4[..., i, :] for i in range(4))
    lin_k = np.where(kpm[..., None], lin_k, 0.0)
    ng = T // G
    grp = lambda t: t.reshape(B, ng, G, t.shape[-1])
    qq, lq, qkk, lk, vg = map(grp, (quad_q, lin_q, quad_k, lin_k, vh))
    kpm_g = kpm.reshape(B, ng, 1, G)
    sim = np.einsum("bgid,bgjd->bgij", qq, qkk) / G
    attn = (1.0 + erf((sim - MU_L) / (STD_L * math.sqrt(2.0)))) * 0.5
    attn = np.where(kpm_g, attn, 0.0)
    quad_out = np.einsum("bgij,bgje->bgie", attn, vg)
    lin_kv = np.einsum("bgnd,bgne->bgde", lk, vg) / T
    lin_out = np.einsum("bgnd,bgde->bgne", lq, lin_kv)
    out = gate * (quad_out + lin_out).reshape(B, T, HID)
    out = (out @ Wo + bo).transpose(0, 2, 1)
    return (out * masks).astype(np.float32)


def kernel(**inputs):
    in_maps, m1 = _host_prep(inputs)
    if in_maps is None:
        return _numpy_fallback(inputs)

    from concourse.bass_utils import run_bass_kernel_spmd

    nc = _get_program()
    core_ids = list(range(8))
    res = run_bass_kernel_spmd(nc, in_maps, core_ids)
    out = np.empty((B, C, T), np.float32)
    for b in range(B):
        # final token mask applied host-side (elementwise, exact)
        out[b] = np.asarray(res.results[b]["out"], dtype=np.float32) \
            * m1[b][None, :]
    return out


if __name__ == "__main__":
    rng = np.random.default_rng(0)
    ins = {
        "q": rng.standard_normal((B, C, T), dtype=np.float32),
        "k": rng.standard_normal((B, C, T), dtype=np.float32),
        "v": rng.standard_normal((B, C, T), dtype=np.float32),
        "masks": rng.integers(0, 2, (B, 1, T)).astype(np.float32),
        "Wg": (rng.standard_normal((C, HID)) * 0.02).astype(np.float32),
        "bg": np.zeros(HID, np.float32),
        "Wv": (rng.standard_normal((C, HID)) * 0.02).astype(np.float32),
        "bv": np.zeros(HID, np.float32),
        "Wqk": (rng.standard_normal((C, QK)) * 0.02).astype(np.float32),
        "bqk": np.zeros(QK, np.float32),
        "gamma": (1 + rng.standard_normal((4, QK)) * 0.02).astype(np.float32),
        "beta": np.zeros((4, QK), np.float32),
        "Wo": (rng.standard_normal((HID, C)) * 0.02).astype(np.float32),
        "bo": np.zeros(C, np.float32),
    }
    got = kernel(**ins)
    exp = _numpy_fallback(ins)
    err = np.abs(got - exp).max() / np.abs(exp).max()
    print("absmax-rel err vs numpy:", err)


# revision 26
# speedup vs baseline: 1.1534x; 1.0057x over previous
"""Trainium2 Bass kernel for nn_MixedChunkAttentionLayer.

Sharding: pure data-parallel over batch — B=8 batches onto 8 NeuronCores,
one batch per core, zero cross-core communication.

Per-core pipeline (batch b, C=256, T=8192, G=128, QK=128, HID=512):
  - InstanceNorm is folded into the projections: bn_stats/bn_aggr + Newton
    rsqrt give rstd r and mean mu per channel; the kernel scales
    Wg/Wqk by r on-device (Wg_s = r*Wg) and computes per-feature bias
    K = (-mu*r)^T W_s via tiny matmuls, so the projections consume the
    RAW bf16 q pieces as the moving operand and the silu ACT applies the
    bias: gate = silu(Wg_s^T q + K). No qn materialization pass.
  - v arrives host-premasked (v*m0, m0=1 where mask==0) which absorbs both
    the quadratic attn j-mask and the linear lin_k n-mask, so vm needs no
    per-group scale and the OffsetScale gammas fold into qsA/qsB:
      qsA = qkT*(g0*g2/G), qsB = qkT*(g1*g3/T)
  - The linear branch collapses into the quadratic one (R = laplace(sim)+S);
    sim and S for each group are produced by ONE matmul with the moving
    operand [qsA_g | qsB_g] packed [128,256], psum [128,1024] per supertile.
  - laplace chain engineered for engine balance: the two PSUM reads run on
    ACT (zzr = D1*sim+D2 -> bf16) and DVE (stl = S+0.5 -> bf16); the rest
    are cheap all-SBUF bf16 DVE ops (w = zzr^2, tt2 = (1+w)*zzr,
    R = 0.5*tanh(AB*tt2)+stl) eligible for the DVE 2x/4x fast modes.
  - z^T[ec] = vm_g[:,ec]^T @ R_g; z = z^T * gateT; out^T = Wo^T z, * m1,
    stored bf16 (host upcasts to f32).

laplace_attn(x) = Phi((x-mu)/sigma) is evaluated as
  0.5*(1 + tanh(zz*(a + b*zz^2))), zz=(x-mu)/sigma
(max abs err 1.8e-4) so every ACT function used (Silu/Tanh/Identity)
lives in the single `silu_and_others` table set — no table reloads.
"""

import math
import sys

if "/opt/trn_rl_repo" not in sys.path:
    sys.path.insert(0, "/opt/trn_rl_repo")

import numpy as np
import ml_dtypes

B, C, T = 8, 256, 8192
G = 128
QK = 128
HID = 512
NG = T // G          # 64 groups
ST = 512             # supertile token count
NST = T // ST        # 16 supertiles
GPS = ST // G        # 4 groups per supertile
NCC = C // 128       # 2 contraction chunks
NHC = HID // 128     # 4 HID chunks
NOC = C // 128       # 2 output-channel chunks

MU_L = math.sqrt(0.5)
STD_L = math.sqrt(0.25 * math.pi)
S1_L = 1.0 / STD_L           # zz = S1*x + C1
C1_L = -MU_L / STD_L
A_C = math.sqrt(2.0 / math.pi)
B_C = A_C * 0.044715
# zzr = D1*x + D2 = sqrt(B/A)*zz ; w = zzr^2 = (B/A)*zz^2
# tanh arg = A*zz*(1+w) = AB_SCALE * (1+w)*zzr
_RBA = math.sqrt(B_C / A_C)
D1_L = S1_L * _RBA
D2_L = C1_L * _RBA
AB_SCALE = A_C / _RBA

_PROG = None  # cached — program is input-independent


def _build_program():
    import concourse.bass as bass
    import concourse.tile as tile
    from concourse import bacc, mybir

    f32 = mybir.dt.float32
    bf16 = mybir.dt.bfloat16
    i32 = mybir.dt.int32
    AF = mybir.ActivationFunctionType
    OP = mybir.AluOpType

    nc = bacc.Bacc("TRN2", target_bir_lowering=False, debug=False, num_devices=8)

    q_d = nc.dram_tensor("q", [C, T], bf16, kind="ExternalInput")
    v_d = nc.dram_tensor("v", [C, T], bf16, kind="ExternalInput")
    wg_d = nc.dram_tensor("wg", [C, HID], bf16, kind="ExternalInput")
    wv_d = nc.dram_tensor("wv", [C, HID], bf16, kind="ExternalInput")
    wqk_d = nc.dram_tensor("wqk", [C, QK], bf16, kind="ExternalInput")
    wo_d = nc.dram_tensor("wo", [HID, C], bf16, kind="ExternalInput")
    gA_d = nc.dram_tensor("gA", [QK, 1], f32, kind="ExternalInput")
    gB_d = nc.dram_tensor("gB", [QK, 1], f32, kind="ExternalInput")
    out_d = nc.dram_tensor("out", [C, T], bf16, kind="ExternalOutput")

    with tile.TileContext(nc) as tc:
        with (
            tc.tile_pool(name="const", bufs=1) as p_const,
            tc.tile_pool(name="qstage", bufs=1) as p_qstage,
            tc.tile_pool(name="stats", bufs=2) as p_stats,
            tc.tile_pool(name="vstage", bufs=16) as p_vstage,
            tc.tile_pool(name="stw", bufs=2) as p_st,          # within-supertile
            tc.tile_pool(name="stx", bufs=8) as p_stx,         # live 2 supertiles
            tc.tile_pool(name="lap", bufs=2) as p_lap,         # laplace temps
            tc.tile_pool(name="carry", bufs=2) as p_carry,     # R across phases
            tc.tile_pool(name="outp", bufs=2) as p_out,
            tc.tile_pool(name="psA", bufs=4, space="PSUM") as psA,
            tc.tile_pool(name="psAttn", bufs=1, space="PSUM") as psAttn,
            tc.tile_pool(name="psZ", bufs=2, space="PSUM") as psZ,
        ):
            # ---------------- constants ----------------
            # weight loads go on the gpsimd HWDGE queue so the sync queue is
            # free for the v supertile loads the PE prologue depends on;
            # wv first — the vh prelude needs it.
            wg_sb = []
            wv_sb = []
            wqk_sb = []
            for cc in range(NCC):
                t_ = p_const.tile([128, HID], bf16, tag=f"wv{cc}", name=f"wv{cc}")
                nc.gpsimd.dma_start(out=t_, in_=wv_d[cc * 128:(cc + 1) * 128, :])
                wv_sb.append(t_)
            gA_sb = p_const.tile([QK, 1], f32, tag="gA")
            nc.gpsimd.dma_start(out=gA_sb, in_=gA_d[:, :])
            gB_sb = p_const.tile([QK, 1], f32, tag="gB")
            nc.gpsimd.dma_start(out=gB_sb, in_=gB_d[:, :])
            wo_sb = []

            def emit_late_weights():
                # emitted after the first vh v-loads so the gpsimd DMA ring
                # serves the PE prologue's v tiles first
                for cc in range(NCC):
                    t_ = p_const.tile([128, HID], bf16, tag=f"wg{cc}",
                                      name=f"wg{cc}")
                    nc.gpsimd.dma_start(
                        out=t_, in_=wg_d[cc * 128:(cc + 1) * 128, :])
                    wg_sb.append(t_)
                    t_ = p_const.tile([128, QK], bf16, tag=f"wqk{cc}",
                                      name=f"wqk{cc}")
                    nc.gpsimd.dma_start(
                        out=t_, in_=wqk_d[cc * 128:(cc + 1) * 128, :])
                    wqk_sb.append(t_)
                for hc in range(NHC):
                    t_ = p_const.tile([128, C], bf16, tag=f"wo{hc}",
                                      name=f"wo{hc}")
                    nc.gpsimd.dma_start(
                        out=t_, in_=wo_d[hc * 128:(hc + 1) * 128, :])
                    wo_sb.append(t_)

            bias_d2 = p_const.tile([128, 1], f32, tag="bias_d2")
            nc.vector.memset(bias_d2, D2_L)
            bias_one = p_const.tile([128, 1], f32, tag="bias_one")
            nc.vector.memset(bias_one, 1.0)

            # ---------------- q staging + stats ----------------
            # q streamed in [128, QP] pieces; the SAME pieces stay resident as
            # the projections' moving operand (instance norm is folded into
            # the weights), so q is read from HBM exactly once.
            NQP = 4
            QP = T // NQP

            def emit_q_stats():
                qpieces = []
                statst = []
                for cc in range(NCC):
                    pieces = []
                    stats = p_stats.tile([128, T // 512, 6], f32,
                                         tag=f"bnstats{cc}", name="stats")
                    for p in range(NQP):
                        qf = p_qstage.tile([128, QP], bf16, tag=f"qf{cc}{p}",
                                           name="qf", bufs=1)
                        # q alternates between the scalar and sync HWDGE
                        # queues for ~2x the issue share — q gates the stats
                        # and therefore the whole projection pipeline.
                        eng = nc.scalar if (cc * NQP + p) % 2 == 0 else nc.sync
                        eng.dma_start(
                            out=qf,
                            in_=q_d[cc * 128:(cc + 1) * 128,
                                    p * QP:(p + 1) * QP],
                        )
                        qfv = qf.rearrange("p (n f) -> p n f", f=512)
                        for n in range(QP // 512):
                            nc.vector.bn_stats(
                                out=stats[:, p * (QP // 512) + n, :],
                                in_=qfv[:, n, :],
                            )
                        pieces.append(qf)
                    qpieces.append(pieces)
                    statst.append(stats)
                return qpieces, statst

            def emit_norm_fold(statst):
                """bn_aggr + Newton rsqrt -> rstd y; scale Wg/Wqk by y and
                build the -mu*r projection biases via tiny matmuls."""
                mvs = []
                for cc in range(NCC):
                    mv = p_stats.tile([128, 2], f32, tag=f"mv{cc}", name="mv")
                    nc.vector.bn_aggr(out=mv, in_=statst[cc])
                    mvs.append(mv)
                s_ = p_stats.tile([128, 2], f32, tag="nt_s", name="s_")
                for cc in range(NCC):
                    nc.vector.tensor_scalar(
                        out=s_[:, cc:cc + 1], in0=mvs[cc][:, 1:2],
                        scalar1=1e-5, scalar2=None, op0=OP.add,
                    )
                t1i = p_stats.tile([128, 2], i32, tag="nt_t1", name="t1i")
                nc.vector.tensor_scalar(
                    out=t1i, in0=s_.bitcast(i32), scalar1=1, scalar2=None,
                    op0=OP.arith_shift_right,
                )
                y0i = p_stats.tile([128, 2], i32, tag="nt_y0", name="y0i")
                nc.vector.tensor_scalar(
                    out=y0i, in0=t1i, scalar1=-1, scalar2=0x5F3759DF,
                    op0=OP.mult, op1=OP.add,
                )
                y = y0i.bitcast(f32)
                for it in range(3):
                    aa = p_stats.tile([128, 2], f32, tag=f"nt_a{it}", name="aa")
                    nc.vector.tensor_mul(out=aa, in0=y, in1=y)
                    nc.vector.tensor_mul(out=aa, in0=aa, in1=s_)
                    nc.vector.tensor_scalar(
                        out=aa, in0=aa, scalar1=-0.5, scalar2=1.5,
                        op0=OP.mult, op1=OP.add,
                    )
                    yn = p_stats.tile([128, 2], f32, tag=f"nt_y{it}", name="yn")
                    nc.vector.tensor_mul(out=yn, in0=y, in1=aa)
                    y = yn
                # murn = -mu * rstd (bf16: matmul operand next to bf16 weights)
                murn = p_stats.tile([128, 2], bf16, tag="nt_mr", name="murn")
                for cc in range(NCC):
                    nc.vector.tensor_scalar(
                        out=murn[:, cc:cc + 1], in0=mvs[cc][:, 0:1],
                        scalar1=y[:, cc:cc + 1], scalar2=-1.0,
                        op0=OP.mult, op1=OP.mult,
                    )
                # scaled weights
                wg_s = []
                wqk_s = []
                for cc in range(NCC):
                    t_ = p_const.tile([128, HID], bf16, tag=f"wgs{cc}",
                                      name=f"wgs{cc}")
                    nc.vector.tensor_scalar(
                        out=t_, in0=wg_sb[cc], scalar1=y[:, cc:cc + 1],
                        scalar2=None, op0=OP.mult,
                    )
                    wg_s.append(t_)
                    t_ = p_const.tile([128, QK], bf16, tag=f"wqks{cc}",
                                      name=f"wqks{cc}")
                    nc.vector.tensor_scalar(
                        out=t_, in0=wqk_sb[cc], scalar1=y[:, cc:cc + 1],
                        scalar2=None, op0=OP.mult,
                    )
                    wqk_s.append(t_)
                # biases K[:, j] = sum_c murn[c] * W_s[c, block_j]
                psK = psA.tile([128, 8], f32, tag="psA", name="psK")
                for hc in range(NHC):
                    for cc in range(NCC):
                        nc.tensor.matmul(
                            psK[:, hc:hc + 1],
                            wg_s[cc][:, hc * 128:(hc + 1) * 128],
                            murn[:, cc:cc + 1],
                            start=(cc == 0), stop=(cc == NCC - 1),
                        )
                for cc in range(NCC):
                    nc.tensor.matmul(
                        psK[:, NHC:NHC + 1], wqk_s[cc][:, :],
                        murn[:, cc:cc + 1],
                        start=(cc == 0), stop=(cc == NCC - 1),
                    )
                biasK = p_stats.tile([128, 8], f32, tag="biasK", name="biasK")
                nc.vector.tensor_copy(out=biasK, in_=psK)
                return wg_s, wqk_s, biasK

            # ---------------- supertile pipeline ----------------
            st_state = {}

            def emit_vh(st):
                # v-side work — independent of the q stats, so it pre-runs
                # and keeps PE busy during the prologue. v is host-premasked.
                t0 = st * ST
                vb = []
                for cc in range(NCC):
                    vb_t = p_vstage.tile([128, ST], bf16, tag="vbf", name="vb_t")
                    nc.gpsimd.dma_start(
                        out=vb_t, in_=v_d[cc * 128:(cc + 1) * 128, t0:t0 + ST]
                    )
                    vb.append(vb_t)
                vm = []
                for g in range(GPS):
                    pv = psA.tile([128, HID], f32, tag="psA", name="pv")
                    for cc in range(NCC):
                        nc.tensor.matmul(
                            pv[:, :],
                            vb[cc][:, g * G:(g + 1) * G],
                            wv_sb[cc][:, :],
                            start=(cc == 0), stop=(cc == NCC - 1),
                        )
                    vm_t = p_stx.tile([128, HID], bf16, tag="vm", name="vm_t",
                                      bufs=40)
                    nc.scalar.activation(out=vm_t, in_=pv, func=AF.Silu)
                    vm.append(vm_t)
                st_state[st] = dict(vm=vm)

            def qmov(qpieces, cc, st):
                p = st // (NQP)
                off = (st % NQP) * ST
                # piece p spans NST//NQP supertiles of 512 tokens
                return qpieces[cc][p][:, off:off + ST]

            def emit_qproj(st, qpieces, wg_s, wqk_s, biasK):
                # qkT = silu(Wqk_s^T q + Kq): [QK, ST]
                pq = psA.tile([128, ST], f32, tag="psA", name="pq")
                for cc in range(NCC):
                    nc.tensor.matmul(
                        pq[:, :], wqk_s[cc][:, :], qmov(qpieces, cc, st),
                        start=(cc == 0), stop=(cc == NCC - 1),
                    )
                qkT = p_st.tile([128, ST], bf16, tag="qkT", name="qkT")
                nc.scalar.activation(
                    out=qkT, in_=pq, func=AF.Silu, bias=biasK[:, NHC:NHC + 1]
                )
                # packed [qsA_g | qsB_g] per group for the merged sim/S matmul
                qsAB = p_st.tile([128, GPS * 2 * G], bf16, tag="qsAB",
                                 name="qsAB")
                qsABv = qsAB.rearrange("p (g two f) -> p g two f", two=2, f=G)
                qkTv = qkT.rearrange("p (g f) -> p g f", f=G)
                nc.vector.tensor_scalar(
                    out=qsABv[:, :, 0, :], in0=qkTv, scalar1=gA_sb,
                    scalar2=None, op0=OP.mult,
                )
                nc.vector.tensor_scalar(
                    out=qsABv[:, :, 1, :], in0=qkTv, scalar1=gB_sb,
                    scalar2=None, op0=OP.mult,
                )
                # gateT = silu(Wg_s^T q + Kg): 4 h-chunks [128h, ST]
                gate = []
                for hc in range(NHC):
                    pg = psA.tile([128, ST], f32, tag="psA", name="pg")
                    for cc in range(NCC):
                        nc.tensor.matmul(
                            pg[:, :],
                            wg_s[cc][:, hc * 128:(hc + 1) * 128],
                            qmov(qpieces, cc, st),
                            start=(cc == 0), stop=(cc == NCC - 1),
                        )
                    g_t = p_stx.tile([128, ST], bf16, tag="gate", name="g_t")
                    nc.scalar.activation(
                        out=g_t, in_=pg, func=AF.Silu,
                        bias=biasK[:, hc:hc + 1],
                    )
                    gate.append(g_t)
                st_state[st].update(qkT=qkT, qsAB=qsAB, gate=gate)

            def emit_attn_find(st, late):
                # ONE matmul per group produces [sim_g | S_g] packed in a
                # [128, 1024] psum; laplace runs on strided views.
                # R' = tanh(arg) + (2S+1) = 2*(laplace+S); the 0.5 is folded
                # into Wo on the host. Chain uses only tensor_tensor /
                # tensor_scalar ops (DVE 2x fast modes; stt has none).
                S = st_state[st]
                psAB = psAttn.tile([128, GPS * 2 * G], f32, tag="psAt",
                                   name="psAB")
                for g in range(GPS):
                    nc.tensor.matmul(
                        psAB[:, g * 2 * G:(g + 1) * 2 * G],
                        S["qkT"][:, g * G:(g + 1) * G],
                        S["qsAB"][:, g * 2 * G:(g + 1) * 2 * G],
                        start=True, stop=True,
                    )
                psv = psAB.rearrange("p (g two f) -> p g two f", two=2, f=G)
                # zzr = D1*sim + D2 (ACT reads PSUM, writes bf16 SBUF)
                zzr = p_lap.tile([128, ST], bf16, tag="zzr", name="zzr")
                nc.scalar.activation(
                    out=zzr, in_=psv[:, :, 0, :], func=AF.Identity,
                    bias=bias_d2, scale=D1_L,
                )
                # stl2 = 2*S + 1 (PSUM read; DVE early, ACT when vh is done)
                stl2 = p_lap.tile([128, ST], bf16, tag="stl", name="stl2")
                if late:
                    nc.scalar.activation(
                        out=stl2, in_=psv[:, :, 1, :], func=AF.Identity,
                        bias=bias_one, scale=2.0,
                    )
                else:
                    nc.vector.tensor_scalar(
                        out=stl2, in0=psv[:, :, 1, :], scalar1=2.0,
                        scalar2=1.0, op0=OP.mult, op1=OP.add,
                    )
                # w = zzr^2 ; u = w*zzr ; tt2 = u + zzr ; th = tanh(AB*tt2)
                w = p_lap.tile([128, ST], bf16, tag="w", name="w")
                nc.vector.tensor_mul(out=w, in0=zzr, in1=zzr)
                u = p_lap.tile([128, ST], bf16, tag="u", name="u")
                nc.vector.tensor_mul(out=u, in0=w, in1=zzr)
                tt2 = p_lap.tile([128, ST], bf16, tag="tt2", name="tt2")
                nc.vector.tensor_add(out=tt2, in0=u, in1=zzr)
                th = p_lap.tile([128, ST], bf16, tag="th", name="th")
                nc.scalar.activation(out=th, in_=tt2, func=AF.Tanh,
                                     scale=AB_SCALE)
                R = p_carry.tile([128, ST], bf16, tag="R", name="R")
                nc.vector.tensor_add(out=R, in0=th, in1=stl2)
                S["R"] = R

            def emit_attn_apply_z(st, ecs):
                # z^T[ec] = sum_g vm_g[:,ec]^T @ R_g; ec-major so each pz bank
                # is consumed right after its 4 MMs
                S = st_state[st]
                z = S.setdefault("z", [])
                for ec in ecs:
                    pz = psZ.tile([128, ST], f32, tag="psZ", name=f"pz{ec}")
                    for g in range(GPS):
                        sl = slice(g * G, (g + 1) * G)
                        nc.tensor.matmul(
                            pz[:, sl],
                            S["vm"][g][:, ec * 128:(ec + 1) * 128],
                            S["R"][:, sl],
                            start=True, stop=True,
                        )
                    z_t = p_out.tile([128, ST], bf16, tag=f"z{ec}", name=f"z{ec}")
                    nc.vector.tensor_mul(out=z_t, in0=pz, in1=S["gate"][ec])
                    z.append(z_t)

            def emit_attn_out(st, late):
                t0 = st * ST
                S = st_state[st]
                z = S["z"]
                # out^T = Wo'^T z (Wo pre-halved on host); the m1 token mask is
                # applied on the host, so this is a pure PSUM->SBUF f32 copy —
                # DVE early, ACT once vh silus are done.
                for oc in range(NOC):
                    po = psA.tile([128, ST], f32, tag="psA", name="po")
                    for hc in range(NHC):
                        nc.tensor.matmul(
                            po[:, :],
                            wo_sb[hc][:, oc * 128:(oc + 1) * 128],
                            z[hc][:, :],
                            start=(hc == 0), stop=(hc == NHC - 1),
                        )
                    ot = p_out.tile([128, ST], bf16, tag="oc", name="ot")
                    if late and oc == 0:
                        nc.scalar.activation(out=ot, in_=po, func=AF.Identity)
                    else:
                        nc.vector.tensor_copy(out=ot, in_=po)
                    nc.sync.dma_start(
                        out=out_d[oc * 128:(oc + 1) * 128, t0:t0 + ST], in_=ot
                    )
                del st_state[st]

            PRE_K = 6
            # vh emission stops at iteration NST-PRE_K; later iterations have
            # spare ACT capacity, so PSUM->SBUF copies flex DVE -> ACT there.
            LATE = NST - PRE_K
            qpieces, statst = emit_q_stats()
            for st in range(3):
                emit_vh(st)
            emit_late_weights()
            for st in range(3, PRE_K):
                emit_vh(st)
            wg_s, wqk_s, biasK = emit_norm_fold(statst)
            # apply(st-1) is emitted BEFORE find(st): engine queues are FIFO
            # in emission order, so this keeps the z-gate multiplies (which
            # free the psZ banks the PE z-matmuls rotate through) ahead of
            # find's laplace chain on the DVE queue. vh sits between the two
            # apply halves so PE has independent matmuls during the psZ
            # bank-recycle wait.
            for st in range(NST):
                emit_qproj(st, qpieces, wg_s, wqk_s, biasK)
                if st >= 1:
                    emit_attn_apply_z(st - 1, ecs=[0, 1])
                if st + PRE_K < NST:
                    emit_vh(st + PRE_K)
                if st >= 1:
                    emit_attn_apply_z(st - 1, ecs=[2, 3])
                    emit_attn_out(st - 1, late=(st - 1 >= LATE - 1))
                emit_attn_find(st, late=(st >= LATE))
            emit_attn_apply_z(NST - 1, ecs=[0, 1, 2, 3])
            emit_attn_out(NST - 1, late=True)

    nc.compile()
    return nc


def _get_program():
    global _PROG
    if _PROG is None:
        _PROG = _build_program()
    return _PROG


def _host_prep(inputs):
    """Build per-core input maps. Returns (in_maps, None) for the fast path
    or (None, reason) when the fast path's preconditions fail."""
    bf = ml_dtypes.bfloat16
    q = np.ascontiguousarray(np.asarray(inputs["q"], dtype=np.float32).astype(bf))
    masks = np.asarray(inputs["masks"], dtype=np.float32)
    for name in ("bg", "bv", "bqk", "bo", "beta"):
        if np.any(np.asarray(inputs[name]) != 0.0):
            return None, f"nonzero {name}"

    gamma = np.asarray(inputs["gamma"], dtype=np.float32)
    gA = (gamma[0] * gamma[2] / G).reshape(QK, 1).astype(np.float32)
    gB = (gamma[1] * gamma[3] / T).reshape(QK, 1).astype(np.float32)
    wg = np.asarray(inputs["Wg"], dtype=np.float32).astype(bf)
    wv = np.asarray(inputs["Wv"], dtype=np.float32).astype(bf)
    wqk = np.asarray(inputs["Wqk"], dtype=np.float32).astype(bf)
    # device computes R' = 2*(laplace+S); the 0.5 is folded into Wo
    wo = (np.asarray(inputs["Wo"], dtype=np.float32) * 0.5).astype(bf)

    # gen_key_padding_mask: all-zero mask batches are reset to ones
    m1 = np.where(masks.sum(axis=(1, 2), keepdims=True) == 0.0, 1.0, masks)
    m1 = m1[:, 0, :].astype(np.float32)          # [B, T]
    m0 = 1.0 - m1                                 # 1 where mask==0
    # premask v: absorbs the attn j-mask and the lin_k n-mask
    vm_host = (np.asarray(inputs["v"], dtype=np.float32)
               * m0[:, None, :]).astype(bf)

    in_maps = []
    for b in range(B):
        in_maps.append({
            "q": q[b],
            "v": np.ascontiguousarray(vm_host[b]),
            "wg": wg, "wv": wv, "wqk": wqk, "wo": wo,
            "gA": gA, "gB": gB,
        })
    return in_maps, m1


def _numpy_fallback(inputs):
    """Exact-semantics fp32 fallback for inputs outside the fast path
    (nonzero biases/beta). Mirrors the reference in numpy."""
    from scipy.special import erf

    def silu(x):
        return x / (1.0 + np.exp(-x))

    q = np.asarray(inputs["q"], np.float32)
    v = np.asarray(inputs["v"], np.float32)
    masks = np.asarray(inputs["masks"], np.float32)
    Wg, bg = np.asarray(inputs["Wg"], np.float32), np.asarray(inputs["bg"], np.float32)
    Wv, bv = np.asarray(inputs["Wv"], np.float32), np.asarray(inputs["bv"], np.float32)
    Wqk, bqk = np.asarray(inputs["Wqk"], np.float32), np.asarray(inputs["bqk"], np.float32)
    gamma, beta = np.asarray(inputs["gamma"], np.float32), np.asarray(inputs["beta"], np.float32)
    Wo, bo = np.asarray(inputs["Wo"], np.float32), np.asarray(inputs["bo"], np.float32)

    all_zero = masks.sum(axis=(1, 2)) == 0.0
    masks = np.where(all_zero[:, None, None], 1.0, masks)
    kpm = masks[:, 0, :] == 0.0
    mu = q.mean(-1, keepdims=True)
    var = q.var(-1, keepdims=True)
    qn = (q - mu) / np.sqrt(var + 1e-5)
    x = qn.transpose(0, 2, 1)
    vt = v.transpose(0, 2, 1)
    gate = silu(x @ Wg + bg)
    vh = silu(vt @ Wv + bv)
    qk = silu(x @ Wqk + bqk)
    qk4 = qk[..., None, :] * gamma + beta
    quad_q, lin_q, quad_k, lin_k = (qk4[..., i, :] for i in range(4))
    lin_k = np.where(kpm[..., None], lin_k, 0.0)
    ng = T // G
    grp = lambda t: t.reshape(B, ng, G, t.shape[-1])
    qq, lq, qkk, lk, vg = map(grp, (quad_q, lin_q, quad_k, lin_k, vh))
    kpm_g = kpm.reshape(B, ng, 1, G)
    sim = np.einsum("bgid,bgjd->bgij", qq, qkk) / G
    attn = (1.0 + erf((sim - MU_L) / (STD_L * math.sqrt(2.0)))) * 0.5
    attn = np.where(kpm_g, attn, 0.0)
    quad_out = np.einsum("bgij,bgje->bgie", attn, vg)
    lin_kv = np.einsum("bgnd,bgne->bgde", lk, vg) / T
    lin_out = np.einsum("bgnd,bgde->bgne", lq, lin_kv)
    out = gate * (quad_out + lin_out).reshape(B, T, HID)
    out = (out @ Wo + bo).transpose(0, 2, 1)
    return (out * masks).astype(np.float32)


def kernel(**inputs):
    in_maps, m1 = _host_prep(inputs)
    if in_maps is None:
        return _numpy_fallback(inputs)

    from concourse.bass_utils import run_bass_kernel_spmd

    nc = _get_program()
    core_ids = list(range(8))
    res = run_bass_kernel_spmd(nc, in_maps, core_ids)
    out = np.empty((B, C, T), np.float32)
    for b in range(B):
        # final token mask applied host-side (elementwise, exact)
        out[b] = np.asarray(res.results[b]["out"], dtype=np.float32) \
            * m1[b][None, :]
    return out


if __name__ == "__main__":
    rng = np.random.default_rng(0)
    ins = {
        "q": rng.standard_normal((B, C, T), dtype=np.float32),
        "k": rng.standard_normal((B, C, T), dtype=np.float32),
        "v": rng.standard_normal((B, C, T), dtype=np.float32),
        "masks": rng.integers(0, 2, (B, 1, T)).astype(np.float32),
        "Wg": (rng.standard_normal((C, HID)) * 0.02).astype(np.float32),
        "bg": np.zeros(HID, np.float32),
        "Wv": (rng.standard_normal((C, HID)) * 0.02).astype(np.float32),
        "bv": np.zeros(HID, np.float32),
        "Wqk": (rng.standard_normal((C, QK)) * 0.02).astype(np.float32),
        "bqk": np.zeros(QK, np.float32),
        "gamma": (1 + rng.standard_normal((4, QK)) * 0.02).astype(np.float32),
        "beta": np.zeros((4, QK), np.float32),
        "Wo": (rng.standard_normal((HID, C)) * 0.02).astype(np.float32),
        "bo": np.zeros(C, np.float32),
    }
    got = kernel(**ins)
    exp = _numpy_fallback(ins)
    err = np.abs(got - exp).max() / np.abs(exp).max()
    print("absmax-rel err vs numpy:", err)
